# revision 5
# baseline (speedup 1.0000x reference)
"""Cross-attention kernel for Trainium2, 8 NeuronCores, data-parallel over batch.

Computes, per batch b (one batch per core):
    q_proj = q[b] @ Wq          [Nq, E]
    k_proj = y[b] @ Wk          [Nk, E]
    v_proj = k_proj @ Wv        [Nk, F]   (faithful quirk: value() of key-projection)
    scores = q_proj @ k_proj.T / sqrt(E)
    out    = softmax(scores, -1) @ v_proj

Device-side layout strategy: all activations are kept "feature-major"
([feature_part, token_free]) so every matmul contracts along the SBUF
partition dim with zero on-device transposes.  The host pre-transposes
q/y once (cheap numpy) when building the per-core input maps.

scoresT [m, n] = (k_projT as lhsT).T-free @ q_projT   -> partition = keys m
exp runs on ScalarE with the 1/sqrt(E) folded into the activation scale;
no max-subtraction is needed (weights are scale 0.02 -> |score| < ~3).
The softmax denominator comes from an extra 1-column matmul against a
ones vector that rides on the same loaded weights (eT block) as the
out-matmuls; the output block is then scaled by the reciprocal.

Matmul dtypes: projections in fp32r (full-rate on TRN2 for free-dim >=
256, ~tf32 accuracy, zero cast cost from the fp32 inputs); attention
matmuls in bf16 (projection outputs are rounded to bf16 on the
PSUM->SBUF copy, halving SBUF so everything stays resident).
"""

import numpy as np
from contextlib import ExitStack

import concourse.bass as bass
import concourse.tile as tile
from concourse import bacc, mybir
from concourse.bass_utils import run_bass_kernel_spmd

P = 128
F32 = mybir.dt.float32
F32R = mybir.dt.float32r
BF16 = mybir.dt.bfloat16

# Problem shapes (hardcoded per contract)
B = 8
NQ = 2048
NK = 2048
D = 1024   # in_q_dim == in_dim
E = 1024   # hid_q == out_dim
F = 1024   # out_dim (v)


def build_program(
    nq=NQ, nk=NK, d=D, e=E, f=F,
    nblk=512,          # query block (columns of q_projT processed per round)
    mblk=512,          # key block for the k-projection phase
    proj_dtype="f32r",  # matmul dtype for the three projections
):
    """Build the single-core Bass program (same program runs SPMD on all cores)."""
    nc = bacc.Bacc(trn_type="TRN2")

    DC = d // P            # contraction chunks for the projections
    EC = e // P
    MC = nk // P           # key chunks
    MB = nk // mblk
    NB = nq // nblk
    NSUB = nblk // P
    FCH = (f + 511) // 512  # 512-wide chunks of the value dim
    fch = [min(512, f - 512 * j) for j in range(FCH)]
    sch = min(512, nblk)   # scores free dim per matmul == nblk (<=512)
    assert nblk <= 512 and mblk <= 512

    pf = F32R if proj_dtype == "f32r" else F32
    qT = nc.dram_tensor("qT", [d, nq], pf, kind="ExternalInput").ap()
    yT = nc.dram_tensor("yT", [d, nk], pf, kind="ExternalInput").ap()
    Wq = nc.dram_tensor("Wq", [d, e], pf, kind="ExternalInput").ap()
    Wk = nc.dram_tensor("Wk", [d, e], pf, kind="ExternalInput").ap()
    Wv = nc.dram_tensor("Wv", [e, f], F32, kind="ExternalInput").ap()
    out = nc.dram_tensor("out", [nq, f], F32, kind="ExternalOutput").ap()

    qT_v = qT.rearrange("(c p) n -> p c n", p=P)     # [P, DC, nq]
    yT_v = yT.rearrange("(c p) n -> p c n", p=P)     # [P, DC, nk]
    Wq_v = Wq.rearrange("(c p) e -> p c e", p=P)     # [P, DC, e]
    Wk_v = Wk.rearrange("(c p) e -> p c e", p=P)
    Wv_v = Wv.rearrange("(c p) f -> p c f", p=P)     # [P, EC, f]
    out_v = out.rearrange("(b p) f -> b p f", p=P)   # [nq//P, P, f]

    def pdt(ap):
        return ap

    with tile.TileContext(nc) as tc, ExitStack() as ctx:
        consts = ctx.enter_context(tc.tile_pool(name="consts", bufs=1))
        staging = ctx.enter_context(tc.tile_pool(name="staging", bufs=2))
        kproj_pool = ctx.enter_context(tc.tile_pool(name="kproj", bufs=1))
        v_pool = ctx.enter_context(tc.tile_pool(name="vproj", bufs=1))
        wq_pool = ctx.enter_context(tc.tile_pool(name="wq", bufs=1))
        psum_a = ctx.enter_context(
            tc.tile_pool(name="psum_a", bufs=3, space="PSUM"))

        ones_bf = consts.tile([P, 1], BF16)
        nc.vector.memset(ones_bf, 1.0)
        zbias = consts.tile([P, 1], F32)
        nc.vector.memset(zbias, 0.0)

        kprojT = kproj_pool.tile([P, EC, nk], BF16)   # [e_part, e_chunk, m]
        v_sb = v_pool.tile([P, MC, f], BF16)          # [m_part, m_chunk, f]
        wq_sb = wq_pool.tile([P, DC, e], pf)

        # ---- Phase 1+2: k-projection, then v-projection (transient weights) --
        with tc.tile_pool(name="wk", bufs=1) as wk_pool, \
             tc.tile_pool(name="wvbf", bufs=1) as wv_pool:
            wk_sb = wk_pool.tile([P, DC, e], pf)
            nc.sync.dma_start(wk_sb, Wk_v)

            # Wv: load fp32 through staging, round to bf16
            wv_bf = wv_pool.tile([P, EC, f], BF16)
            for j in range(FCH):
                st = staging.tile([P, EC, 512], F32, tag="stage")
                nc.sync.dma_start(st[:, :, :fch[j]],
                                  Wv_v[:, :, 512 * j: 512 * j + fch[j]])
                nc.vector.tensor_copy(wv_bf[:, :, 512 * j: 512 * j + fch[j]],
                                      st[:, :, :fch[j]])

            # k_projT[e, m] = sum_d Wk[d, e].T @ yT[d, m]
            for mb in range(MB):
                yt = staging.tile([P, DC, mblk], pf, tag="stage")
                nc.sync.dma_start(yt, yT_v[:, :, mb * mblk:(mb + 1) * mblk])
                for ei in range(EC):
                    ps = psum_a.tile([P, 512], F32, tag="psa", name="psa")[:, :mblk]
                    for di in range(DC):
                        nc.tensor.matmul(
                            ps,
                            lhsT=pdt(wk_sb[:, di, ei * P:(ei + 1) * P]),
                            rhs=pdt(yt[:, di, :]),
                            start=(di == 0), stop=(di == DC - 1))
                    nc.vector.tensor_copy(
                        kprojT[:, ei, mb * mblk:(mb + 1) * mblk], ps)

            # prefetch Wq during the (DMA-free) v phase
            nc.sync.dma_start(wq_sb, Wq_v)

            # v[m, f] = sum_e k_projT[e, m].T @ Wv[e, f]   (bf16)
            for mi in range(MC):
                for j in range(FCH):
                    ps = psum_a.tile([P, 512], F32, tag="psa", name="psa")[:, :fch[j]]
                    for ei in range(EC):
                        nc.tensor.matmul(
                            ps,
                            lhsT=kprojT[:, ei, mi * P:(mi + 1) * P],
                            rhs=wv_bf[:, ei, 512 * j: 512 * j + fch[j]],
                            start=(ei == 0), stop=(ei == EC - 1))
                    nc.vector.tensor_copy(v_sb[:, mi, 512 * j: 512 * j + fch[j]], ps)

        # ---- Phase 3: attention, blocked over queries ----
        qproj_pool = ctx.enter_context(tc.tile_pool(name="qproj", bufs=2))
        eT_pool = ctx.enter_context(tc.tile_pool(name="eT", bufs=2))
        out_pool = ctx.enter_context(tc.tile_pool(name="outsb", bufs=2))
        small = ctx.enter_context(tc.tile_pool(name="small", bufs=4))
        psum_o = ctx.enter_context(
            tc.tile_pool(name="psum_o", bufs=4, space="PSUM"))
        psum_s = ctx.enter_context(
            tc.tile_pool(name="psum_s", bufs=1, space="PSUM"))

        for nb in range(NB):
            qt = staging.tile([P, DC, nblk], pf, tag="stage")
            nc.sync.dma_start(qt, qT_v[:, :, nb * nblk:(nb + 1) * nblk])

            # q_projT[e, n_blk]  (bf16)
            qp = qproj_pool.tile([P, EC, nblk], BF16)
            for ei in range(EC):
                ps = psum_a.tile([P, 512], F32, tag="psa", name="psa")[:, :nblk]
                for di in range(DC):
                    nc.tensor.matmul(
                        ps,
                        lhsT=pdt(wq_sb[:, di, ei * P:(ei + 1) * P]),
                        rhs=pdt(qt[:, di, :]),
                        start=(di == 0), stop=(di == DC - 1))
                nc.vector.tensor_copy(qp[:, ei, :], ps)

            # eT[m, n_blk] = exp(scoresT / sqrt(E))
            eT = eT_pool.tile([P, MC, nblk], BF16)
            for mi in range(MC):
                ps = psum_a.tile([P, 512], F32, tag="psa", name="psa")[:, :sch]
                for ei in range(EC):
                    nc.tensor.matmul(
                        ps,
                        lhsT=kprojT[:, ei, mi * P:(mi + 1) * P],
                        rhs=qp[:, ei, :],
                        start=(ei == 0), stop=(ei == EC - 1))
                nc.scalar.activation(
                    eT[:, mi, :], ps,
                    mybir.ActivationFunctionType.Exp,
                    bias=zbias, scale=1.0 / float(np.sqrt(e)))

            # out[n, f] = (eT.T @ v) / (eT.T @ 1)
            for ns in range(NSUB):
                pos = [psum_o.tile([P, 512], F32, tag="pso", name="pso")[:, :fch[j]]
                       for j in range(FCH)]
                pss = psum_s.tile([P, 1], F32, tag="pss", name="pss")
                for mi in range(MC):
                    lhsT_e = eT[:, mi, ns * P:(ns + 1) * P]
                    for j in range(FCH):
                        nc.tensor.matmul(
                            pos[j], lhsT=lhsT_e,
                            rhs=v_sb[:, mi, 512 * j: 512 * j + fch[j]],
                            start=(mi == 0), stop=(mi == MC - 1))
                    nc.tensor.matmul(
                        pss, lhsT=lhsT_e, rhs=ones_bf,
                        start=(mi == 0), stop=(mi == MC - 1))
                rec = small.tile([P, 1], F32)
                nc.vector.reciprocal(rec, pss)
                ob = out_pool.tile([P, f], F32)
                for j in range(FCH):
                    nc.vector.tensor_scalar_mul(
                        ob[:, 512 * j: 512 * j + fch[j]], pos[j], rec)
                nc.sync.dma_start(out_v[nb * NSUB + ns], ob)

    nc.compile()
    return nc


_CACHE = {}


def kernel(q, y, Wq, Wk, Wv):
    q = np.asarray(q, dtype=np.float32)
    y = np.asarray(y, dtype=np.float32)
    Wq = np.ascontiguousarray(np.asarray(Wq, dtype=np.float32))
    Wk = np.ascontiguousarray(np.asarray(Wk, dtype=np.float32))
    Wv = np.ascontiguousarray(np.asarray(Wv, dtype=np.float32))

    if "nc" not in _CACHE:
        _CACHE["nc"] = build_program()
    nc = _CACHE["nc"]

    in_maps = []
    for b in range(B):
        in_maps.append({
            "qT": np.ascontiguousarray(q[b].T),
            "yT": np.ascontiguousarray(y[b].T),
            "Wq": Wq, "Wk": Wk, "Wv": Wv,
        })
    res = run_bass_kernel_spmd(nc, in_maps, core_ids=list(range(B)))
    return np.stack([res.results[b]["out"] for b in range(B)], axis=0)
